# revision 18
# baseline (speedup 1.0000x reference)
"""Trainium2 Bass kernel for nn_ChannelSelfAttention.

Reference computation (per batch sample b):
    xt   = x[b].T                          # [C, L]
    q    = xt @ Wq.T + bq                  # [C, H]
    kv   = xt @ Wkv.T + bkv                # [C, 2H] -> k, v
    attn = (q * H**-0.5) @ k.T             # [C, C]  (no softmax)
    y    = attn @ v                        # [C, H]
    g    = mean(y, axis=-1)                # [C]
    out[b] = x[b] * g[None, :]             # [L, C]

No softmax -> everything after qkv is LINEAR, so the [C,C] attention
matrix is never materialized:

    g[c] = sum_h q''[c,h] * kvb[h]
    kvb[h] = sum_d k[d,h] * vbar[d],  vbar[d] = sum_h' v[d,h']

with scale/H folded into Wq/bq on the host (q'' = q * scale / H).

Sharding: data-parallel over B across 8 cores (4 samples per core);
weights replicated.  All HBM I/O is bf16 (host casts): 17.5 MiB per
core, and the DMA stream runs at the per-NC HBM/SDMA wall, so the
schedule keeps the sync ring 100% fed and the tail short:

  - Constants ride the scalar (ACT HWDGE) queue; bulk x/wT/out on sync
    with 8KB/6KB descriptors (j=16 rows per partition per chunk; the
    DMA engine assignment splits descriptor blocks evenly, so counts
    must stay 128-divisible).
  - wT in two per-chunk tiles interleaved with sample 0's x halves so
    the first qkv matmuls start ~2us earlier.
  - Loads issue before any store so store-gate semaphore waits on the
    SP sequencer can't delay a load.
  - PE warm-up is one continuous ~6us run of wide junk matmuls so the
    clock p-state is ramped when the first real matmul issues.
  - PE emission is software-pipelined: qkv(b+1) half-0 between qkv(b)
    drain and sample b's tail matmuls, half-1 after.
  - vbar reduces read qkv directly from PSUM (no wait on the ACT copy)
    and are emitted ahead of the previous sample's gate multiplies so
    Tile's readiness scheduler never parks them behind 4.5us of DVE.
  - Per-sample tail chain: [ACT qkv copy || DVE vbar] -> PE (kvb via a
    stride-0 broadcast rhs of vbar, + q transposes, one shared PSUM
    tile) -> one ACT copy -> PE g matmul -> ACT g copy -> DVE gate
    multiplies, chunked 4x512KB with a store issued per chunk; the
    last sample splits its chunks across DVE and the idle GPSIMD
    engine so the final stores enqueue earlier.
"""

import numpy as np
import ml_dtypes

import concourse.bass as bass
import concourse.mybir as mybir
import concourse.tile as tile
from concourse import bacc
from concourse.bass_utils import run_bass_kernel_spmd

B, L, C, H = 32, 4096, 256, 64
N_CORES = 8
B_LOC = B // N_CORES          # samples per core
P = 128                       # SBUF partitions
JC = 16                       # L-rows per partition per chunk (8KB bf16 descs)
NCH = L // (P * JC)           # l-chunks per sample (2)
GC = C // P                   # c-groups (2)
TH = 3 * H                    # 192 = q|k|v
BF16 = mybir.dt.bfloat16
F32 = mybir.dt.float32
SCALE = float(H) ** -0.5
BF = ml_dtypes.bfloat16
N_WARM = 14                   # PE warm-up junk matmuls (512 cols each)
WCOL = 512


def _emit(tc: "tile.TileContext", x_d, wT_d, bias_d, id_d, out_d) -> None:
    nc = tc.nc
    with (
        tc.tile_pool(name="singles", bufs=1) as singles,
        tc.tile_pool(name="xin", bufs=B_LOC) as xin,
        tc.tile_pool(name="xout", bufs=6) as xout,
        tc.tile_pool(name="small", bufs=2) as small,
        tc.tile_pool(name="psW", bufs=1, space="PSUM") as psW,
        tc.tile_pool(name="psA", bufs=2, space="PSUM") as psA,
        tc.tile_pool(name="psA2", bufs=2, space="PSUM") as psA2,
        tc.tile_pool(name="psM", bufs=2, space="PSUM") as psM,
        tc.tile_pool(name="psG", bufs=1, space="PSUM") as psG,
    ):
        # ---- constants on the scalar (ACT HWDGE) queue: tiny, and the
        # SWDGE path's ~1us-per-DMA descriptor gen lands them too late ----
        cb = singles.tile([1, P + TH], BF16)             # ones | (bq*s/H)|bkv
        nc.scalar.dma_start(out=cb, in_=bias_d[:])
        ones1 = cb[:, 0:P]
        bias_sb = cb[:, P : P + TH]
        ident = singles.tile([P, P], BF16)
        nc.scalar.dma_start(out=ident, in_=id_d[:])

        # ---- PE warm-up: one continuous run of wide junk matmuls so the
        # HAM clock-gate reaches full p-state before real work ----
        scratch = singles.tile([P, P + WCOL], BF16)
        nc.vector.memset(scratch, 0.0)
        psj = psW.tile([P, WCOL], F32, tag="warm", name="psj")
        for _ in range(N_WARM):
            nc.tensor.matmul(psj, lhsT=scratch[:, 0:P],
                             rhs=scratch[:, P : P + WCOL])

        # ---- bulk loads on the sync ring, in consumption order ----
        wT_src = wT_d[:].rearrange("(n p j) h -> p n (j h)", p=P, j=JC)
        x_srcs = [x_d[b].rearrange("(n p j) c -> n p (j c)", p=P, j=JC)
                  for b in range(B_LOC)]
        out_dsts = [out_d[b].rearrange("(n p j) c -> n p (j c)", p=P, j=JC)
                    for b in range(B_LOC)]
        # wT in two per-chunk tiles interleaved with sample 0's x halves:
        # qkv(0) half-0 can start right after 1.75MB instead of 2.5MB.
        wts = [singles.tile([P, JC * TH], BF16, name=f"wt{h}")
               for h in range(NCH)]
        xs = [[xin.tile([P, JC * C], BF16, tag=f"x{h}", name=f"x_b{b}_h{h}")
               for h in range(NCH)] for b in range(B_LOC)]
        for h in range(NCH):
            nc.sync.dma_start(
                out=wts[h],
                in_=wT_src[:, h : h + 1].rearrange("p n x -> p (n x)"),
            )
            nc.sync.dma_start(
                out=xs[0][h],
                in_=x_srcs[0][h : h + 1].rearrange("n p x -> p (n x)"),
            )
        def _pslice(ap, p0, pn):
            """Partition-range slice [p0, p0+pn) of a 2D [p, x] AP."""
            return bass.AP(
                tensor=ap.tensor,
                offset=ap.offset + p0 * ap.ap[0][0],
                ap=[[ap.ap[0][0], pn], list(ap.ap[1])],
            )

        for b in range(1, B_LOC):
            for h in range(NCH):
                src = x_srcs[b][h : h + 1].rearrange("n p x -> p (n x)")
                if b == 2:
                    # lane-15 underloading: 120- and 8-descriptor DMAs are
                    # split over DMA lanes 0-7 only (descriptor blocks are
                    # dealt over the largest lane count in {16,8,4,2,1}
                    # dividing the count), so the intermittently-slow lane
                    # 15 skips ~2MB of mid-stream bytes.
                    dst = bass.AP(tensor=xs[b][h].tensor,
                                  offset=xs[b][h].offset,
                                  ap=[list(xs[b][h].ap[0]),
                                      list(xs[b][h].ap[1])])
                    nc.sync.dma_start(out=_pslice(dst, 0, 120),
                                      in_=_pslice(src, 0, 120))
                    nc.sync.dma_start(out=_pslice(dst, 120, 8),
                                      in_=_pslice(src, 120, 8))
                else:
                    nc.sync.dma_start(out=xs[b][h], in_=src)

        def qkv_bias(b):
            """Open sample b's PSUM accumulation groups with the bias."""
            pq = [psA.tile([P, TH], F32, tag="qkv0", name=f"pq0_b{b}"),
                  psA2.tile([P, TH], F32, tag="qkv1", name=f"pq1_b{b}")]
            for g in range(GC):
                nc.tensor.matmul(
                    pq[g], lhsT=ones1, rhs=bias_sb, start=True, stop=False,
                )
            return pq

        def qkv_half(b, h, pq):
            """Chunk matmuls for l-chunk h of sample b (x-stationary)."""
            for j in range(JC):
                for g in range(GC):
                    nc.tensor.matmul(
                        pq[g],
                        lhsT=xs[b][h][:, j * C + g * P : j * C + (g + 1) * P],
                        rhs=wts[h][:, j * TH : (j + 1) * TH],
                        start=False,
                        stop=(h == NCH - 1 and j == JC - 1),
                    )

        def qkv_copy(b, pq):
            qkv_sb = small.tile([P, GC, TH], BF16, tag="qkv_sb")
            for g in range(GC):
                nc.scalar.copy(qkv_sb[:, g], pq[g])
            return qkv_sb

        def vbar_stage(b, pq):
            """vbar[d] = sum_h v[d,h], read straight from the qkv PSUM."""
            vbar_sb = small.tile([P, GC, 1], BF16, tag="vbar")
            with nc.allow_low_precision(reason="bf16 vbar feeds bf16 matmul"):
                for g in range(GC):
                    nc.vector.tensor_reduce(
                        out=vbar_sb[:, g], in_=pq[g][:, 2 * H : TH],
                        axis=mybir.AxisListType.X, op=mybir.AluOpType.add,
                    )
            return vbar_sb

        def chain_stage(b, qkv_sb, vbar_sb):
            """kvb + q^T -> one PSUM tile -> one copy -> g matmul."""
            ps_m = psM.tile([H, P + C], F32, tag="m")
            # kvb[h] (broadcast along 128 free cols): lhsT = k-section,
            # rhs = vbar as a stride-0 broadcast row
            for g in range(GC):
                vb_bc = bass.AP(
                    tensor=vbar_sb.tensor,
                    offset=vbar_sb.offset + g * vbar_sb.ap[1][0],
                    ap=[list(vbar_sb.ap[0]), [0, P]],
                )
                nc.tensor.matmul(
                    ps_m[:, 0:P], lhsT=qkv_sb[:, g, H : 2 * H], rhs=vb_bc,
                    start=(g == 0), stop=(g == GC - 1),
                )
            # q^T [64, 256] via PE matmul against the identity
            for g in range(GC):
                nc.tensor.matmul(
                    ps_m[:, P + g * P : P + (g + 1) * P],
                    lhsT=qkv_sb[:, g, 0:H], rhs=ident,
                )
            m_sb = small.tile([H, P + C], BF16, tag="m_sb")
            nc.scalar.copy(m_sb, ps_m)
            # g[c] = sum_h kvb[h] qT[h, c], landing on all 128 partitions
            ps_g = psG.tile([P, C], F32, tag="g")
            nc.tensor.matmul(ps_g, lhsT=m_sb[:, 0:P], rhs=m_sb[:, P : P + C])
            g_sb = small.tile([P, C], BF16, tag="g_sb")
            nc.scalar.copy(g_sb, ps_g)
            return g_sb

        def gate_store(b, h, g_sb, eng):
            """1MB gate multiply + store for half h of sample b.  One DVE
            op per half: tensor_tensor pays ~400ns fixed per instruction
            on top of 2x-rate streaming, so bigger ops are cheaper."""
            g_bc = bass.AP(
                tensor=g_sb.tensor,
                offset=g_sb.offset,
                ap=[list(g_sb.ap[0]), [0, JC], list(g_sb.ap[1])],
            )
            o_t = xout.tile([P, JC * C], BF16, tag="o", name=f"o_b{b}_h{h}")
            eng.tensor_tensor(
                out=o_t.rearrange("p (j c) -> p j c", c=C),
                in0=xs[b][h].rearrange("p (j c) -> p j c", c=C),
                in1=g_bc,
                op=mybir.AluOpType.mult,
            )
            nc.sync.dma_start(
                out=out_dsts[b][h].rearrange("p x -> p x"),
                in_=o_t,
            )

        # ---- software-pipelined emission over samples ----
        pq = qkv_bias(0)
        qkv_half(0, 0, pq)
        qkv_half(0, 1, pq)
        cur_sb = qkv_copy(0, pq)
        cur_vb = vbar_stage(0, pq)
        for b in range(B_LOC):
            nxt_pq = None
            if b + 1 < B_LOC:
                nxt_pq = qkv_bias(b + 1)
                qkv_half(b + 1, 0, nxt_pq)
            g_sb = chain_stage(b, cur_sb, cur_vb)
            if b + 1 < B_LOC:
                qkv_half(b + 1, 1, nxt_pq)
                cur_sb = qkv_copy(b + 1, nxt_pq)
                cur_vb = vbar_stage(b + 1, nxt_pq)
            if b < B_LOC - 1:
                for h in range(NCH):
                    gate_store(b, h, g_sb, nc.vector)
            else:
                # last sample: DVE takes half 1 first (its store leads in
                # the ring FIFO), the idle GPSIMD engine computes half 0
                # concurrently so the final stores enqueue earlier
                gate_store(b, 1, g_sb, nc.vector)
                gate_store(b, 0, g_sb, nc.gpsimd)


def build():
    nc = bacc.Bacc(
        "TRN2", target_bir_lowering=False, debug=False, num_devices=N_CORES
    )
    x_d = nc.dram_tensor("x", [B_LOC, L, C], BF16, kind="ExternalInput")
    wT_d = nc.dram_tensor("wT", [L, TH], BF16, kind="ExternalInput")
    bias_d = nc.dram_tensor("bias", [1, P + TH], BF16, kind="ExternalInput")
    id_d = nc.dram_tensor("ident", [P, P], BF16, kind="ExternalInput")
    out_d = nc.dram_tensor("out", [B_LOC, L, C], BF16, kind="ExternalOutput")
    with tile.TileContext(nc) as tc:
        _emit(tc, x_d, wT_d, bias_d, id_d, out_d)
    nc.compile()
    return nc


_nc_cache = None


def _get_nc():
    global _nc_cache
    if _nc_cache is None:
        _nc_cache = build()
    return _nc_cache


def make_in_maps(x, Wq, bq, Wkv, bkv):
    x_bf = np.asarray(x, dtype=np.float32).astype(BF)
    qs = SCALE / H                      # fold attn scale AND mean-over-H into q
    wT = np.ascontiguousarray(
        np.concatenate(
            [np.asarray(Wq, np.float32) * qs, np.asarray(Wkv, np.float32)],
            axis=0,
        ).T.astype(BF)
    )
    bias = np.concatenate(
        [np.asarray(bq, np.float32) * qs, np.asarray(bkv, np.float32)]
    )[None].astype(BF)
    ident = np.eye(P, dtype=BF)
    cb = np.concatenate([np.ones((1, P), dtype=BF), bias], axis=1)
    return [
        {
            "x": np.ascontiguousarray(x_bf[i * B_LOC : (i + 1) * B_LOC]),
            "wT": wT,
            "bias": cb,
            "ident": ident,
        }
        for i in range(N_CORES)
    ]


def run(inputs, **spmd_kwargs):
    """Run on hardware; returns (full_output, BassKernelResults)."""
    nc = _get_nc()
    in_maps = make_in_maps(**inputs)
    res = run_bass_kernel_spmd(nc, in_maps, list(range(N_CORES)), **spmd_kwargs)
    out = np.concatenate([r["out"] for r in res.results], axis=0)
    return np.asarray(out).astype(np.float32), res


def kernel(**inputs) -> np.ndarray:
    out, _ = run(inputs)
    return out


# revision 19
# speedup vs baseline: 1.1395x; 1.1395x over previous
"""Trainium2 Bass kernel for nn_ChannelSelfAttention.

Reference computation (per batch sample b):
    xt   = x[b].T                          # [C, L]
    q    = xt @ Wq.T + bq                  # [C, H]
    kv   = xt @ Wkv.T + bkv                # [C, 2H] -> k, v
    attn = (q * H**-0.5) @ k.T             # [C, C]  (no softmax)
    y    = attn @ v                        # [C, H]
    g    = mean(y, axis=-1)                # [C]
    out[b] = x[b] * g[None, :]             # [L, C]

No softmax -> everything after qkv is LINEAR, so the [C,C] attention
matrix is never materialized:

    g[c] = sum_h q''[c,h] * kvb[h]
    kvb[h] = sum_d k[d,h] * vbar[d],  vbar[d] = sum_h' v[d,h']

with scale/H folded into Wq/bq on the host (q'' = q * scale / H).

Sharding: data-parallel over B across 8 cores (4 samples per core);
weights replicated.  All HBM I/O is bf16 (host casts): 17.5 MiB per
core, and the DMA stream runs at the per-NC HBM/SDMA wall, so the
schedule keeps the sync ring 100% fed and the tail short:

  - Constants ride the scalar (ACT HWDGE) queue; bulk x/wT/out on sync
    with 8KB/6KB descriptors (j=16 rows per partition per chunk; the
    DMA engine assignment splits descriptor blocks evenly, so counts
    must stay 128-divisible).
  - wT in two per-chunk tiles interleaved with sample 0's x halves so
    the first qkv matmuls start ~2us earlier.
  - Loads issue before any store so store-gate semaphore waits on the
    SP sequencer can't delay a load.
  - PE warm-up is one continuous ~6us run of wide junk matmuls so the
    clock p-state is ramped when the first real matmul issues.
  - PE emission is software-pipelined: qkv(b+1) half-0 between qkv(b)
    drain and sample b's tail matmuls, half-1 after.
  - vbar reduces read qkv directly from PSUM (no wait on the ACT copy)
    and are emitted ahead of the previous sample's gate multiplies so
    Tile's readiness scheduler never parks them behind 4.5us of DVE.
  - Per-sample tail chain: [ACT qkv copy || DVE vbar] -> PE (kvb via a
    stride-0 broadcast rhs of vbar, + q transposes, one shared PSUM
    tile) -> one ACT copy -> PE g matmul -> ACT g copy -> DVE gate
    multiplies, chunked 4x512KB with a store issued per chunk; the
    last sample splits its chunks across DVE and the idle GPSIMD
    engine so the final stores enqueue earlier.
"""

import numpy as np
import ml_dtypes

import concourse.bass as bass
import concourse.mybir as mybir
import concourse.tile as tile
from concourse import bacc
from concourse.bass_utils import run_bass_kernel_spmd

B, L, C, H = 32, 4096, 256, 64
N_CORES = 8
B_LOC = B // N_CORES          # samples per core
P = 128                       # SBUF partitions
JC = 16                       # L-rows per partition per chunk (8KB bf16 descs)
NCH = L // (P * JC)           # l-chunks per sample (2)
GC = C // P                   # c-groups (2)
TH = 3 * H                    # 192 = q|k|v
BF16 = mybir.dt.bfloat16
F32 = mybir.dt.float32
SCALE = float(H) ** -0.5
BF = ml_dtypes.bfloat16
N_WARM = 14                   # PE warm-up junk matmuls (512 cols each)
WCOL = 512


def _emit(tc: "tile.TileContext", x_d, wT_d, bias_d, id_d, out_d) -> None:
    nc = tc.nc
    with (
        tc.tile_pool(name="singles", bufs=1) as singles,
        tc.tile_pool(name="xin", bufs=B_LOC) as xin,
        tc.tile_pool(name="xout", bufs=12) as xout,
        tc.tile_pool(name="small", bufs=2) as small,
        tc.tile_pool(name="psW", bufs=1, space="PSUM") as psW,
        tc.tile_pool(name="psA", bufs=2, space="PSUM") as psA,
        tc.tile_pool(name="psA2", bufs=2, space="PSUM") as psA2,
        tc.tile_pool(name="psM", bufs=2, space="PSUM") as psM,
        tc.tile_pool(name="psG", bufs=1, space="PSUM") as psG,
    ):
        # ---- constants on the scalar (ACT HWDGE) queue: tiny, and the
        # SWDGE path's ~1us-per-DMA descriptor gen lands them too late ----
        cb = singles.tile([1, P + TH], BF16)             # ones | (bq*s/H)|bkv
        nc.scalar.dma_start(out=cb, in_=bias_d[:])
        ones1 = cb[:, 0:P]
        bias_sb = cb[:, P : P + TH]
        ident = singles.tile([P, P], BF16)
        nc.scalar.dma_start(out=ident, in_=id_d[:])

        # ---- PE warm-up: one continuous run of wide junk matmuls so the
        # HAM clock-gate reaches full p-state before real work ----
        scratch = singles.tile([P, P + WCOL], BF16)
        nc.vector.memset(scratch, 0.0)
        psj = psW.tile([P, WCOL], F32, tag="warm", name="psj")
        for _ in range(N_WARM):
            nc.tensor.matmul(psj, lhsT=scratch[:, 0:P],
                             rhs=scratch[:, P : P + WCOL])

        # ---- bulk loads on the sync ring, in consumption order ----
        wT_src = wT_d[:].rearrange("(n p j) h -> p n (j h)", p=P, j=JC)
        x_srcs = [x_d[b].rearrange("(n p j) c -> n p (j c)", p=P, j=JC)
                  for b in range(B_LOC)]
        out_dsts = [out_d[b].rearrange("(n p j) c -> n p (j c)", p=P, j=JC)
                    for b in range(B_LOC)]
        # wT in two per-chunk tiles interleaved with sample 0's x halves:
        # qkv(0) half-0 can start right after 1.75MB instead of 2.5MB.
        wts = [singles.tile([P, JC * TH], BF16, name=f"wt{h}")
               for h in range(NCH)]
        xs = [[xin.tile([P, JC * C], BF16, tag=f"x{h}", name=f"x_b{b}_h{h}")
               for h in range(NCH)] for b in range(B_LOC)]
        for h in range(NCH):
            nc.sync.dma_start(
                out=wts[h],
                in_=wT_src[:, h : h + 1].rearrange("p n x -> p (n x)"),
            )
            nc.sync.dma_start(
                out=xs[0][h],
                in_=x_srcs[0][h : h + 1].rearrange("n p x -> p (n x)"),
            )
        def _pslice(ap, p0, pn):
            """Partition-range slice [p0, p0+pn) of a 2D [p, x] AP."""
            return bass.AP(
                tensor=ap.tensor,
                offset=ap.offset + p0 * ap.ap[0][0],
                ap=[[ap.ap[0][0], pn], list(ap.ap[1])],
            )

        for b in range(1, B_LOC):
            for h in range(NCH):
                src = x_srcs[b][h : h + 1].rearrange("n p x -> p (n x)")
                if b == 2:
                    # lane-15 underloading: 120- and 8-descriptor DMAs are
                    # split over DMA lanes 0-7 only (descriptor blocks are
                    # dealt over the largest lane count in {16,8,4,2,1}
                    # dividing the count), so the intermittently-slow lane
                    # 15 skips ~2MB of mid-stream bytes.
                    dst = bass.AP(tensor=xs[b][h].tensor,
                                  offset=xs[b][h].offset,
                                  ap=[list(xs[b][h].ap[0]),
                                      list(xs[b][h].ap[1])])
                    nc.sync.dma_start(out=_pslice(dst, 0, 120),
                                      in_=_pslice(src, 0, 120))
                    nc.sync.dma_start(out=_pslice(dst, 120, 8),
                                      in_=_pslice(src, 120, 8))
                else:
                    nc.sync.dma_start(out=xs[b][h], in_=src)

        def qkv_bias(b):
            """Open sample b's PSUM accumulation groups with the bias."""
            pq = [psA.tile([P, TH], F32, tag="qkv0", name=f"pq0_b{b}"),
                  psA2.tile([P, TH], F32, tag="qkv1", name=f"pq1_b{b}")]
            for g in range(GC):
                nc.tensor.matmul(
                    pq[g], lhsT=ones1, rhs=bias_sb, start=True, stop=False,
                )
            return pq

        def qkv_half(b, h, pq):
            """Chunk matmuls for l-chunk h of sample b (x-stationary)."""
            for j in range(JC):
                for g in range(GC):
                    nc.tensor.matmul(
                        pq[g],
                        lhsT=xs[b][h][:, j * C + g * P : j * C + (g + 1) * P],
                        rhs=wts[h][:, j * TH : (j + 1) * TH],
                        start=False,
                        stop=(h == NCH - 1 and j == JC - 1),
                    )

        def qkv_copy(b, pq):
            qkv_sb = small.tile([P, GC, TH], BF16, tag="qkv_sb")
            for g in range(GC):
                nc.scalar.copy(qkv_sb[:, g], pq[g])
            return qkv_sb

        def vbar_stage(b, pq):
            """vbar[d] = sum_h v[d,h], read straight from the qkv PSUM."""
            vbar_sb = small.tile([P, GC, 1], BF16, tag="vbar")
            with nc.allow_low_precision(reason="bf16 vbar feeds bf16 matmul"):
                for g in range(GC):
                    nc.vector.tensor_reduce(
                        out=vbar_sb[:, g], in_=pq[g][:, 2 * H : TH],
                        axis=mybir.AxisListType.X, op=mybir.AluOpType.add,
                    )
            return vbar_sb

        def chain_stage(b, qkv_sb, vbar_sb):
            """kvb + q^T -> one PSUM tile -> one copy -> g matmul."""
            ps_m = psM.tile([H, P + C], F32, tag="m")
            # kvb[h] (broadcast along 128 free cols): lhsT = k-section,
            # rhs = vbar as a stride-0 broadcast row
            for g in range(GC):
                vb_bc = bass.AP(
                    tensor=vbar_sb.tensor,
                    offset=vbar_sb.offset + g * vbar_sb.ap[1][0],
                    ap=[list(vbar_sb.ap[0]), [0, P]],
                )
                nc.tensor.matmul(
                    ps_m[:, 0:P], lhsT=qkv_sb[:, g, H : 2 * H], rhs=vb_bc,
                    start=(g == 0), stop=(g == GC - 1),
                )
            # q^T [64, 256] via PE matmul against the identity
            for g in range(GC):
                nc.tensor.matmul(
                    ps_m[:, P + g * P : P + (g + 1) * P],
                    lhsT=qkv_sb[:, g, 0:H], rhs=ident,
                )
            m_sb = small.tile([H, P + C], BF16, tag="m_sb")
            nc.scalar.copy(m_sb, ps_m)
            # g[c] = sum_h kvb[h] qT[h, c], landing on all 128 partitions
            ps_g = psG.tile([P, C], F32, tag="g")
            nc.tensor.matmul(ps_g, lhsT=m_sb[:, 0:P], rhs=m_sb[:, P : P + C])
            g_sb = small.tile([P, C], BF16, tag="g_sb")
            nc.scalar.copy(g_sb, ps_g)
            return g_sb

        def gate_store(b, h, jh, g_sb, eng):
            """512KB gate multiply + store for quarter (h, jh) of sample b."""
            g_bc = bass.AP(
                tensor=g_sb.tensor,
                offset=g_sb.offset,
                ap=[list(g_sb.ap[0]), [0, JC // 2], list(g_sb.ap[1])],
            )
            half = JC // 2 * C
            o_t = xout.tile([P, half], BF16, tag="o", name=f"o_b{b}_h{h}j{jh}")
            eng.tensor_tensor(
                out=o_t.rearrange("p (j c) -> p j c", c=C),
                in0=xs[b][h][:, jh * half : (jh + 1) * half]
                .rearrange("p (j c) -> p j c", c=C),
                in1=g_bc,
                op=mybir.AluOpType.mult,
            )
            nc.sync.dma_start(
                out=bass.AP(
                    tensor=out_dsts[b][h].tensor,
                    offset=out_dsts[b][h].offset + jh * half,
                    ap=[list(out_dsts[b][h].ap[0]), [1, half]],
                ),
                in_=o_t,
            )

        # ---- software-pipelined emission over samples ----
        pq = qkv_bias(0)
        qkv_half(0, 0, pq)
        qkv_half(0, 1, pq)
        cur_sb = qkv_copy(0, pq)
        cur_vb = vbar_stage(0, pq)
        for b in range(B_LOC):
            nxt_pq = None
            if b + 1 < B_LOC:
                nxt_pq = qkv_bias(b + 1)
                qkv_half(b + 1, 0, nxt_pq)
            g_sb = chain_stage(b, cur_sb, cur_vb)
            if b + 1 < B_LOC:
                qkv_half(b + 1, 1, nxt_pq)
                cur_sb = qkv_copy(b + 1, nxt_pq)
                cur_vb = vbar_stage(b + 1, nxt_pq)
            if b < B_LOC - 1:
                for h in range(NCH):
                    for jh in range(2):
                        gate_store(b, h, jh, g_sb, nc.vector)
            else:
                # last sample: DVE takes half 1 first (its stores lead in
                # the ring FIFO), the idle GPSIMD engine computes half 0
                # concurrently so the final stores enqueue ~2.5us earlier
                for jh in range(2):
                    gate_store(b, 1, jh, g_sb, nc.vector)
                for jh in range(2):
                    gate_store(b, 0, jh, g_sb, nc.gpsimd)


def build():
    nc = bacc.Bacc(
        "TRN2", target_bir_lowering=False, debug=False, num_devices=N_CORES
    )
    x_d = nc.dram_tensor("x", [B_LOC, L, C], BF16, kind="ExternalInput")
    wT_d = nc.dram_tensor("wT", [L, TH], BF16, kind="ExternalInput")
    bias_d = nc.dram_tensor("bias", [1, P + TH], BF16, kind="ExternalInput")
    id_d = nc.dram_tensor("ident", [P, P], BF16, kind="ExternalInput")
    out_d = nc.dram_tensor("out", [B_LOC, L, C], BF16, kind="ExternalOutput")
    with tile.TileContext(nc) as tc:
        _emit(tc, x_d, wT_d, bias_d, id_d, out_d)
    nc.compile()
    return nc


_nc_cache = None


def _get_nc():
    global _nc_cache
    if _nc_cache is None:
        _nc_cache = build()
    return _nc_cache


def make_in_maps(x, Wq, bq, Wkv, bkv):
    x_bf = np.asarray(x, dtype=np.float32).astype(BF)
    qs = SCALE / H                      # fold attn scale AND mean-over-H into q
    wT = np.ascontiguousarray(
        np.concatenate(
            [np.asarray(Wq, np.float32) * qs, np.asarray(Wkv, np.float32)],
            axis=0,
        ).T.astype(BF)
    )
    bias = np.concatenate(
        [np.asarray(bq, np.float32) * qs, np.asarray(bkv, np.float32)]
    )[None].astype(BF)
    ident = np.eye(P, dtype=BF)
    cb = np.concatenate([np.ones((1, P), dtype=BF), bias], axis=1)
    return [
        {
            "x": np.ascontiguousarray(x_bf[i * B_LOC : (i + 1) * B_LOC]),
            "wT": wT,
            "bias": cb,
            "ident": ident,
        }
        for i in range(N_CORES)
    ]


def run(inputs, **spmd_kwargs):
    """Run on hardware; returns (full_output, BassKernelResults)."""
    nc = _get_nc()
    in_maps = make_in_maps(**inputs)
    res = run_bass_kernel_spmd(nc, in_maps, list(range(N_CORES)), **spmd_kwargs)
    out = np.concatenate([r["out"] for r in res.results], axis=0)
    return np.asarray(out).astype(np.float32), res


def kernel(**inputs) -> np.ndarray:
    out, _ = run(inputs)
    return out
